# revision 33
# baseline (speedup 1.0000x reference)
"""Segment-mean (MessageAggregator) kernel for 8 Trainium2 NeuronCores.

Problem: N=2,097,152 events x D=128 float32 messages, segment ids in
[0, 65536). Output: (unique_ids=arange(65536), mean[65536, 128]).

Strategy
--------
Sharding: events are sharded across the 8 cores BY SEGMENT RANGE: core c
receives exactly the events whose id is in [c*8192, (c+1)*8192).  Each core
then owns a disjoint 8192-segment slice of the output, so no inter-core
reduction is needed; the host just concatenates the 8 slices.

Device kernel (per core, identical program):
  - Message rows (129 f32: 128 message + 1 count flag) are DMA'd (HWDGE,
    sequential) into SBUF staging buffers laid out [128 part, NQ*tpb, 192]
    (cols 129:192 pre-zeroed once).  Each block region holds tpb*128 rows:
    the first half are scatter slots, the second half are same-segment
    partner rows (or zero pads).
  - One strided DVE add per block folds partner rows into their slots
    (PAIR-COMBINING): sums and counts both accumulate because the count
    flag is loaded data (1.0 real / 0.0 pad).  This halves scatter volume.
  - gpsimd.dma_scatter_add (custom SWDGE ucode, `mlp` library) performs
    out[idx, :] += row into a DRAM table [8193, 192] (row 8192 = dump row),
    accumulating per-segment sums AND counts in fp32.  Scatters are
    pipelined with prepare_only + trigger_dma and spread over 4 SWDGE
    queues (disjoint segment quarters).
  - After all scatters, the table is read back 512 segments at a time and
    DVE computes mean = sums * recip(max(count, 1)), stored to the output.

Correctness constraints (scatter RMW is NOT atomic across DMA engines):
  - No duplicate indices inside one scatter instruction: the host sorts
    events by id, pairs a segment's copies two-per-slot, and stripes the
    slots round-robin across the queue's blocks (#blocks >= max slots
    per segment).
  - Between scatter instructions, RMW must not overlap: each queue's
    scatters are chained on their completion semaphore, and the 4 queues
    cover DISJOINT segment quarters (block b covers quarter b%NQ), so
    concurrent chains never touch the same table row (except the dump
    row, which is never read).
"""

import os
import sys

import numpy as np

if "/opt/trn_rl_repo" not in sys.path:
    sys.path.insert(0, "/opt/trn_rl_repo")

from contextlib import ExitStack

import concourse.bacc as bacc
import concourse.mybir as mybir
from concourse.bass_utils import run_bass_kernel_spmd

N_CORES = 8
U_TOTAL = 65536
U_PER_CORE = U_TOTAL // N_CORES  # 8192
D = 128
DC = D + 1  # message + count-flag columns (loaded from DRAM)
ELEM = 192  # 128 sums + 1 count + 63 pad (elem bytes must be %256==0)
NQ = 4  # SWDGE queues = concurrent scatter chains (disjoint seg quarters)
U_PER_Q = U_PER_CORE // NQ  # 2048 segments per queue
TPB = 16  # t-slots per block
LVL = 2  # combine levels: each scatter slot folds up to 2**LVL copies
SCAT_T = TPB >> LVL  # scatter-slot t-groups per block (2)
SLOTS = SCAT_T * 128  # scatter slots per block (256)
BCAP = TPB * 128  # source rows per block (1024)
TBL_ROWS = U_PER_CORE + 1  # + dump row for padding events
DUMP = U_PER_CORE  # dump-row index

# Results of the last device run (for test harness introspection).
LAST_RESULT = {}


def _cdiv(a, b):
    return -(-a // b)


def build_core_program(nbq: int, blk_nidx: list[int]):
    """Bass program for one core.

    nbq: blocks per queue; global block b (0..NQ*nbq-1) is queue b%NQ,
    group (within-queue index) b//NQ.  Block b holds BCAP source rows and
    scatters exactly blk_nidx[b] combined slots (same constants on every
    core).  Group g's NQ blocks are loaded with ONE contiguous DMA.
    """
    nb = NQ * nbq
    assert len(blk_nidx) == nb and 0 < max(blk_nidx) <= SLOTS
    nc = bacc.Bacc(
        "TRN2",
        target_bir_lowering=False,
        debug=False,
        dynamic_dma_scratch_size=32768,  # 2048-descriptor SWDGE rings
        num_swdge_queues=NQ,
    )
    f32 = mybir.dt.float32
    i16 = mybir.dt.int16

    msgs = nc.dram_tensor("msgs", [nb * BCAP, DC], f32, kind="ExternalInput")
    idxw = nc.dram_tensor("idxw", [nb, 128, SLOTS // 16], i16, kind="ExternalInput")
    # ExternalOutput => runtime/PJRT zero-initializes (donated zero buffer).
    table = nc.dram_tensor("table", [TBL_ROWS, ELEM], f32, kind="ExternalOutput")
    out = nc.dram_tensor("out", [U_PER_CORE, D], f32, kind="ExternalOutput")

    NBUF = 3  # staging (group) buffers
    RBW = 4  # table tiles per readback DMA
    NT2 = U_PER_CORE // (128 * RBW)  # 16 readback iterations

    with ExitStack() as ctx:
        aug = [
            ctx.enter_context(
                nc.sbuf_tensor(f"aug{k}", [128, NQ * TPB, ELEM], f32)
            )
            for k in range(NBUF)
        ]
        idxs = [
            ctx.enter_context(
                nc.sbuf_tensor(f"idx{k}", [128, NQ, SLOTS // 16], i16)
            )
            for k in range(NBUF)
        ]
        rb = [
            ctx.enter_context(nc.sbuf_tensor(f"rb{k}", [128, RBW, ELEM], f32))
            for k in range(2)
        ]
        cnt = [
            ctx.enter_context(nc.sbuf_tensor(f"cnt{k}", [128, RBW], f32))
            for k in range(2)
        ]
        rec = [
            ctx.enter_context(nc.sbuf_tensor(f"rec{k}", [128, RBW], f32))
            for k in range(2)
        ]
        mean = [
            ctx.enter_context(nc.sbuf_tensor(f"mean{k}", [128, RBW, D], f32))
            for k in range(2)
        ]
        # Per-buffer-slot DMA semaphores: at most ONE outstanding DMA per
        # semaphore at any time, so every wait threshold is unambiguous
        # regardless of cross-DMA completion ordering.
        s_init = ctx.enter_context(nc.semaphore("s_init"))
        s_lmsg = [
            ctx.enter_context(nc.semaphore(f"s_lmsg{k}")) for k in range(NBUF)
        ]
        s_lidx = [
            ctx.enter_context(nc.semaphore(f"s_lidx{k}")) for k in range(NBUF)
        ]
        s_comb = [
            ctx.enter_context(nc.semaphore(f"s_comb{k}")) for k in range(NBUF)
        ]
        s_prep = ctx.enter_context(nc.semaphore("s_prep"))
        s_scat = [ctx.enter_context(nc.semaphore(f"s_scat{q}")) for q in range(NQ)]
        s_cnt = ctx.enter_context(nc.semaphore("s_cnt"))
        s_rb = [ctx.enter_context(nc.semaphore(f"s_rb{j}")) for j in range(2)]
        s_div = ctx.enter_context(nc.semaphore("s_div"))
        s_out = [ctx.enter_context(nc.semaphore(f"s_out{j}")) for j in range(2)]
        block = ctx.enter_context(nc.Block())

        @block.vector
        def _(v):
            # Cols 129:192 are never written again: zero them once.
            for k in range(NBUF):
                v.memset(aug[k][:, :, DC:ELEM], 0.0).then_inc(s_init, 1)
            # Tree-combine each group's blocks: LVL halving folds merge up
            # to 2**LVL same-segment copies into each scatter slot (slot j
            # sources rows j + k*SLOTS).  Count flags are loaded data, so
            # counts combine too.
            incs_per_group = NQ * LVL
            for g in range(nbq):
                k = g % NBUF
                v.wait_ge(s_lmsg[k], 16 * (g // NBUF + 1))
                done = incs_per_group * (g // NBUF)
                for lv in range(LVL):
                    h = TPB >> (lv + 1)  # 4, then 2
                    if lv > 0:
                        # RAW on the halved region between levels.
                        v.wait_ge(s_comb[k], done + lv * NQ)
                    for q in range(NQ):
                        v.tensor_tensor(
                            out=aug[k][:, q * TPB : q * TPB + h, 0:DC],
                            in0=aug[k][:, q * TPB : q * TPB + h, 0:DC],
                            in1=aug[k][:, q * TPB + h : q * TPB + 2 * h, 0:DC],
                            op=mybir.AluOpType.add,
                        ).then_inc(s_comb[k], 1)
            # Divide phase: mean = sums * recip(max(count, 1)).
            for t in range(NT2):
                v.wait_ge(s_rb[t % 2], 16 * (t // 2 + 1))
                if t >= 2:
                    v.wait_ge(s_out[t % 2], 16 * ((t - 2) // 2 + 1))
                # Same-engine RAW hazards need sem waits (deep DVE pipeline).
                v.tensor_scalar_max(
                    cnt[t % 2][:, :], rb[t % 2][:, :, D], 1.0
                ).then_inc(s_cnt, 1)
                v.wait_ge(s_cnt, 2 * t + 1)
                v.reciprocal(rec[t % 2][:, :], cnt[t % 2][:, :]).then_inc(s_cnt, 1)
                v.wait_ge(s_cnt, 2 * t + 2)
                op = None
                for a in range(RBW):
                    op = v.tensor_scalar_mul(
                        mean[t % 2][:, a, :],
                        rb[t % 2][:, a, 0:D],
                        rec[t % 2][:, a : a + 1],
                    )
                op.then_inc(s_div, 1)

        @block.scalar
        def _(sc):
            # idx loads (one DMA per group of NQ blocks) on the ACT ring.
            for g in range(nbq):
                k = g % NBUF
                if g >= NBUF:
                    for q in range(NQ):
                        sc.wait_ge(s_scat[q], 16 * (g - NBUF + 1))
                iv = idxw[g * NQ : (g + 1) * NQ, :, :].rearrange("g p s -> p g s")
                sc.dma_start(idxs[k][:, :, :], iv).then_inc(s_lidx[k], 16)

        @block.sync
        def _(s):
            for g in range(nbq):
                k = g % NBUF
                if g >= NBUF:
                    # group slot is free once all NQ scatters of group
                    # g-NBUF finished.
                    for q in range(NQ):
                        s.wait_ge(s_scat[q], 16 * (g - NBUF + 1))
                mview = msgs[
                    g * NQ * BCAP : (g + 1) * NQ * BCAP, :
                ].rearrange("(t p) d -> p t d", p=128)
                s.dma_start(aug[k][:, :, 0:DC], mview).then_inc(s_lmsg[k], 16)
            # Readback + store, software-pipelined with the divide on DVE.
            # Tile t covers segments [512t, 512t+512) = quarter t//RBW,
            # final once that queue's chain is done.
            R = 128 * RBW
            for t in range(NT2 + 2):
                if t < NT2:
                    s.wait_ge(s_scat[t // RBW], 16 * nbq)
                    if t >= 2:
                        s.wait_ge(s_div, t - 1)
                    rbv = table[t * R : (t + 1) * R, :].rearrange(
                        "(a p) c -> p a c", p=128
                    )
                    s.dma_start(rb[t % 2][:, :, :], rbv).then_inc(s_rb[t % 2], 16)
                if t >= 2:
                    tt = t - 2
                    s.wait_ge(s_div, tt + 1)
                    ov = out[tt * R : (tt + 1) * R, :].rearrange(
                        "(a p) d -> p a d", p=128
                    )
                    s.dma_start(ov, mean[tt % 2][:, :, :]).then_inc(
                        s_out[tt % 2], 16
                    )

        @block.gpsimd
        def _(gp):
            gp.wait_ge(s_init, NBUF)
            for b in range(nb):
                q = b % NQ
                g = b // NQ
                k = g % NBUF
                n = blk_nidx[b]
                assert n > 0  # _prep_inputs guarantees >=16
                gr = _cdiv(n, 128)
                gi = _cdiv(n, 16)
                # Descriptor generation (prep) overlaps in-flight DMAs;
                # the trigger serializes RMW DMAs within each queue chain.
                gp.wait_ge(s_lidx[k], 16 * (g // NBUF + 1))
                gp.dma_scatter_add(
                    table[:, :],
                    aug[k][:, q * TPB : q * TPB + gr, :],
                    idxs[k][:, q, 0:gi],
                    n,
                    n,
                    ELEM,
                    prepare_only=True,
                    sem=s_scat[q],
                    queue_num=q,
                ).then_inc(s_prep, 1)
                gp.wait_ge(s_prep, b + 1)
                # combined data ready (combine waited on the msg load)
                gp.wait_ge(s_comb[k], NQ * LVL * (g // NBUF + 1))
                if g > 0:
                    gp.wait_ge(s_scat[q], 16 * g)
                gp.trigger_dma(count=1, queue_num=q)

    nc.finalize()
    return nc


def _prep_inputs(node_ids: np.ndarray, messages: np.ndarray):
    """Shard events by segment range and build per-core device inputs.

    Within a core, events are split by segment QUARTER (the NQ scatter
    queues).  Within a quarter, events are sorted by id and PAIRED: slot
    s holds up to two copies of one segment (rows s and s + SLOTS of the
    block).  Slots are striped round-robin across the queue's nbq blocks
    so no block holds the same segment twice.  Per-block slot counts are
    equalized across cores (dump-row pads) so one program serves all 8
    cores.
    """
    ids = np.asarray(node_ids).astype(np.int64, copy=False).ravel()
    messages = np.ascontiguousarray(np.asarray(messages, dtype=np.float32))

    order = np.argsort(ids, kind="stable")  # sorted by id
    seg_counts = np.bincount(ids, minlength=U_TOTAL)
    fold = 1 << LVL  # copies folded per scatter slot
    slot_counts = (seg_counts + fold - 1) // fold
    q_slots = slot_counts.reshape(N_CORES * NQ, U_PER_Q).sum(axis=1)

    nbq = max(
        3,
        int(_cdiv(int(q_slots.max()), SLOTS)),
        int(slot_counts.max()),
    )
    nb = NQ * nbq

    # Per-(core, global block) slot counts; unified across cores below.
    percore_cnt = np.zeros((N_CORES, nb), np.int64)
    for c in range(N_CORES):
        for q in range(NQ):
            m = int(q_slots[c * NQ + q])
            base = np.full(nbq, m // nbq, np.int64)
            base[: m % nbq] += 1
            percore_cnt[c, q::NQ] = base
    # >=16 slots per block so every block scatters and bumps its chain sem.
    blk_nidx = np.maximum(percore_cnt.max(axis=0), 16)
    npc = nb * BCAP

    # Per-event slot assignment (vectorized over all events):
    #   within its (core, quarter): slot_local = slot_counts cumsum offset
    #   + (copy_index // 2); copy parity selects scatter row vs partner.
    slot_of_seg = np.concatenate([[0], np.cumsum(slot_counts)])  # global
    qid = ids // U_PER_Q  # 0..31 (core*NQ + q)
    q_slot_base = np.concatenate([[0], np.cumsum(q_slots)])  # global slots
    sid = ids[order]  # sorted ids
    copy_idx = np.arange(len(sid)) - np.concatenate(
        [[0], np.cumsum(seg_counts)]
    )[sid]  # 0..c-1 within segment
    slot_global = slot_of_seg[sid] + copy_idx // fold
    parity = copy_idx % fold  # sub-row within the slot's fold tree
    # slot index within the quarter
    slot_in_q = slot_global - q_slot_base[qid[order]]

    in_maps = []
    for c in range(N_CORES):
        msg = np.zeros((npc, DC), np.float32)
        ix = np.full((npc,), DUMP, np.int16)  # pads -> dump row
        lo = np.searchsorted(sid, c * U_PER_CORE)
        hi = np.searchsorted(sid, (c + 1) * U_PER_CORE)
        sel = order[lo:hi]
        s_q = slot_in_q[lo:hi]
        par = parity[lo:hi]
        qq = (sid[lo:hi] // U_PER_Q) % NQ
        # slot s of quarter q -> block (s % nbq)*NQ + q, position s // nbq
        blk = (s_q % nbq) * NQ + qq
        pos = s_q // nbq
        row = blk * BCAP + pos + par * SLOTS
        msg[row, 0:D] = messages[sel]
        msg[row, D] = 1.0
        ix[blk * BCAP + pos] = (sid[lo:hi] - c * U_PER_CORE).astype(np.int16)
        # ucode layout: index i of a block lives at [partition i%16, col
        # i//16], replicated across the 8 GPSIMD cores (8x16=128 parts).
        ixs = ix.reshape(nb, BCAP)[:, :SLOTS]
        ixw = ixs.reshape(nb, SLOTS // 16, 16).transpose(0, 2, 1)
        ixw = np.ascontiguousarray(np.tile(ixw, (1, 8, 1)))
        in_maps.append({"msgs": msg, "idxw": ixw})
    return in_maps, nbq, [int(x) for x in blk_nidx]


def kernel(node_ids: np.ndarray, messages: np.ndarray):
    in_maps, nbq, blk_nidx = _prep_inputs(node_ids, messages)
    nc = build_core_program(nbq, blk_nidx)
    trace = bool(int(os.environ.get("KERNEL_TRACE", "0")))
    if trace:
        try:  # axon NTFF hook is not present in all containers
            from antenv.axon_hooks import get_axon_ntff_profile_hook  # noqa: F401
        except ImportError:
            trace = False
    res = run_bass_kernel_spmd(
        nc,
        in_maps,
        core_ids=list(range(N_CORES)),
        trace=trace,
    )
    LAST_RESULT["res"] = res
    LAST_RESULT["nbq"] = nbq
    LAST_RESULT["blk_nidx"] = blk_nidx
    mean = np.concatenate(
        [res.results[c]["out"] for c in range(N_CORES)], axis=0
    ).astype(np.float32)
    unique_ids = np.arange(U_TOTAL, dtype=np.asarray(node_ids).dtype)
    return unique_ids, mean
